# revision 1
# baseline (speedup 1.0000x reference)
"""Mamba-1 style selective scan on 8 Trainium2 NeuronCores.

Sharding: core c -> (batch b = c//2, D-half h = c%2).  Each core receives
x[b] with its local 512 channels permuted to the front (weights permuted to
match), computes y^T[512, T] for its channels, host reassembles.

On-chip layout: partitions = (d_sub in 0..7) x (n in 0..15) "groups" of
8 channels x 16 states; free dim = time (chunks of TC).  The recurrence
   s_t = A_bar*s + (A_bar-1)/A * x_t * B_t,   y_t = sum_n s_t*C_t + D*x
runs as: PE replicates dt/x across state-partitions and contracts y over n;
ACT evaluates exp/softplus; DVE builds the scan inputs and runs the
hardware linear scan (tensor_tensor_scan) along the time axis.
"""

import sys

import numpy as np

sys.path.insert(0, "/opt/trn_rl_repo")

import ml_dtypes

import concourse.bacc as bacc
import concourse.mybir as mybir
import concourse.tile as tile
from concourse.bass_utils import run_bass_kernel_spmd

B, T, D, N, R = 4, 4096, 1024, 16, 64
NCORES = 8
DH = D // 2            # channels per core
TC = 1024              # time chunk for DVE ops
PH = 512               # PSUM half (one bank of f32)
NCH = T // TC
NDT = DH // 128        # 128-channel tiles per core (4)
NG = DH * N // 128     # (d,n) partition groups per core (64)
GPD = NG // NDT        # groups per d-tile (16)
F32 = mybir.dt.float32
BF16 = mybir.dt.bfloat16
SC_BF16 = True
AL = mybir.AluOpType
AF = mybir.ActivationFunctionType

_CACHE = {}


def _patch_act_tables():
    """Make the act-table pass pick natural_log_exp_and_others for Exp+Ln
    (same table indices; strip Exp/Ln from the single-func tables so the
    combined one is the only candidate -> no per-chunk LUT reload ping-pong)."""
    import concourse.bacc as _bacc
    from concourse.hw_specs import get_activation_tables as _orig

    def patched(arch):
        t = _orig(arch)
        exp = mybir.ActivationFunctionType.Exp
        ln = mybir.ActivationFunctionType.Ln
        for name, fns in t.items():
            if name != "natural_log_exp_and_others":
                fns.discard(exp)
                fns.discard(ln)
        return t

    _bacc.get_activation_tables = patched


def _build_program():
    _patch_act_tables()
    nc = bacc.Bacc(
        "TRN2",
        target_bir_lowering=False,
        debug=False,
        num_devices=NCORES,
    )

    x_d = nc.dram_tensor("x", [T, D], F32, kind="ExternalInput")
    st_d = nc.dram_tensor("state_r", [128, NG], F32, kind="ExternalInput")
    a_d = nc.dram_tensor("a_rep", [128, NG], F32, kind="ExternalInput")
    ia_d = nc.dram_tensor("inva_rep", [128, NG], F32, kind="ExternalInput")
    wb_d = nc.dram_tensor("wb_t", [D, N], F32, kind="ExternalInput")
    wc_d = nc.dram_tensor("wc_t", [D, N], F32, kind="ExternalInput")
    w1_d = nc.dram_tensor("wdt1_t", [D, R], F32, kind="ExternalInput")
    w2_d = nc.dram_tensor("wdt2_t", [R, DH], F32, kind="ExternalInput")
    bd_d = nc.dram_tensor("bdt2", [128, NDT], F32, kind="ExternalInput")
    dsk_d = nc.dram_tensor("dskdiag", [128, NDT * 128], F32,
                           kind="ExternalInput")
    sel8_d = nc.dram_tensor("sel128", [128, GPD * 128], F32, kind="ExternalInput")
    seln_d = nc.dram_tensor("seln", [N, 128], F32, kind="ExternalInput")
    nsum_d = nc.dram_tensor("nsum128", [128, GPD * 128],
                            BF16 if SC_BF16 else F32, kind="ExternalInput")
    ident_d = nc.dram_tensor("ident", [128, 128], F32, kind="ExternalInput")
    y_d = nc.dram_tensor("yT", [DH, T], F32, kind="ExternalOutput")

    with tile.TileContext(nc) as tc:
        _body(tc, x_d, st_d, a_d, ia_d, wb_d, wc_d, w1_d, w2_d, bd_d,
              dsk_d, sel8_d, seln_d, nsum_d, ident_d, y_d)

    nc.compile()
    return nc


def _body(tc, x_d, st_d, a_d, ia_d, wb_d, wc_d, w1_d, w2_d, bd_d,
          dsk_d, sel8_d, seln_d, nsum_d, ident_d, y_d):
    nc = tc.nc
    KD = D // 128  # k-tiles over full D for the projections (8)

    with (
        tc.tile_pool(name="const", bufs=1) as const,
        tc.tile_pool(name="xload", bufs=3) as xload,
        tc.tile_pool(name="xt", bufs=2 * KD + 1) as xtp,
        tc.tile_pool(name="proj", bufs=2) as projp,
        tc.tile_pool(name="dtp", bufs=NDT) as dtp,
        tc.tile_pool(name="dtep", bufs=1) as dtep,
        tc.tile_pool(name="rep", bufs=2) as repp,
        tc.tile_pool(name="work", bufs=2) as workp,
        tc.tile_pool(name="scan", bufs=3) as scanp,
        tc.tile_pool(name="yout", bufs=2) as youtp,
        tc.tile_pool(name="ps_t", bufs=1, space="PSUM") as ps_t,
        tc.tile_pool(name="ps_proj", bufs=1, space="PSUM") as ps_proj,
        tc.tile_pool(name="ps_rep", bufs=4, space="PSUM") as ps_rep,
        tc.tile_pool(name="ps_y", bufs=2, space="PSUM") as ps_y,
    ):
        # ---- constants / small state ----
        ident = const.tile([128, 128], F32)
        nc.gpsimd.dma_start(ident, ident_d[:, :])
        sel128 = const.tile([128, GPD, 128], F32)
        nc.gpsimd.dma_start(sel128, sel8_d.ap().rearrange("k (s p) -> k s p", s=GPD))
        seln = const.tile([N, 128], F32)
        nc.gpsimd.dma_start(seln, seln_d[:, :])
        nsum128 = const.tile([128, GPD, 128], BF16 if SC_BF16 else F32)
        nc.gpsimd.dma_start(nsum128, nsum_d.ap().rearrange("k (s p) -> k s p", s=GPD))
        arep = const.tile([128, NG], F32)
        nc.gpsimd.dma_start(arep, a_d[:, :])
        iarep = const.tile([128, NG], F32)
        nc.gpsimd.dma_start(iarep, ia_d[:, :])
        bdt2 = const.tile([128, NDT], F32)
        nc.gpsimd.dma_start(bdt2, bd_d[:, :])
        dskdiag = const.tile([128, NDT, 128], F32)
        nc.gpsimd.dma_start(
            dskdiag, dsk_d.ap().rearrange("k (d p) -> k d p", d=NDT))
        carry = const.tile([128, NG], F32)
        nc.gpsimd.dma_start(carry, st_d[:, :])

        wb = const.tile([128, KD, N], F32)
        nc.gpsimd.dma_start(wb, wb_d.ap().rearrange("(k p) n -> p k n", p=128))
        wc = const.tile([128, KD, N], F32)
        nc.gpsimd.dma_start(wc, wc_d.ap().rearrange("(k p) n -> p k n", p=128))
        w1 = const.tile([128, KD, R], F32)
        nc.gpsimd.dma_start(w1, w1_d.ap().rearrange("(k p) r -> p k r", p=128))
        w2 = const.tile([R, DH], F32)
        nc.gpsimd.dma_start(w2, w2_d[:, :])

        def frontend(ch):
            t0 = ch * TC
            # ---- load x chunk and transpose: xt[db] = x[t0:t0+TC, :].T ----
            xt = [xtp.tile([128, TC], F32, tag="xt", name=f"xt{db}")
                  for db in range(KD)]
            for tp2 in range(TC // 256):
                xl = []
                for j in range(2):
                    tt = 2 * tp2 + j
                    xld = xload.tile([128, D], F32, tag="xld", name=f"xld{j}")
                    nc.sync.dma_start(
                        xld, x_d[t0 + tt * 128: t0 + (tt + 1) * 128, :])
                    xl.append(xld)
                for db in range(KD):
                    pt = ps_t.tile([128, 256], F32, tag="tp")
                    for j in range(2):
                        nc.tensor.transpose(
                            pt[:, j * 128:(j + 1) * 128],
                            xl[j][:, db * 128:(db + 1) * 128], ident
                        )
                    nc.scalar.copy(
                        xt[db][:, tp2 * 256:(tp2 + 1) * 256], pt)

            # ---- projections over full D: xr[r,t], Bt[n,t], Ct[n,t] ----
            xr = projp.tile([R, TC], F32)
            bt = projp.tile([N, TC], F32)
            ct = projp.tile([N, TC], F32)
            brep = repp.tile([128, TC], F32)
            crep = repp.tile([128, TC], BF16 if SC_BF16 else F32)
            for hf in range(TC // PH):
                hs = slice(hf * PH, (hf + 1) * PH)
                pxr = ps_proj.tile([R, PH], F32, tag="proj")
                for k in range(KD):
                    nc.tensor.matmul(pxr, w1[:, k, :], xt[k][:, hs],
                                     start=(k == 0), stop=(k == KD - 1))
                nc.scalar.copy(xr[:, hs], pxr)
                pb = ps_proj.tile([N, PH], F32, tag="proj")
                for k in range(KD):
                    nc.tensor.matmul(pb, wb[:, k, :], xt[k][:, hs],
                                     start=(k == 0), stop=(k == KD - 1))
                nc.scalar.copy(bt[:, hs], pb)
                pc = ps_proj.tile([N, PH], F32, tag="proj")
                for k in range(KD):
                    nc.tensor.matmul(pc, wc[:, k, :], xt[k][:, hs],
                                     start=(k == 0), stop=(k == KD - 1))
                nc.scalar.copy(ct[:, hs], pc)
                prb = ps_rep.tile([128, PH], F32, tag="rep")
                nc.tensor.matmul(prb, seln, bt[:, hs], start=True, stop=True)
                nc.scalar.copy(brep[:, hs], prb)
                prc = ps_rep.tile([128, PH], F32, tag="rep")
                nc.tensor.matmul(prc, seln, ct[:, hs], start=True, stop=True)
                nc.scalar.copy(crep[:, hs], prc)

            # ---- dt per d-tile: softplus(W2 @ xr + b) ----
            dts = []
            for dtl in range(NDT):
                dtt = dtp.tile([128, TC], F32, tag="dtt", name=f"dtt{dtl}")
                for hf in range(TC // PH):
                    hs = slice(hf * PH, (hf + 1) * PH)
                    pdt = ps_proj.tile([128, PH], F32, tag="proj")
                    nc.tensor.matmul(pdt, w2[:, dtl * 128:(dtl + 1) * 128],
                                     xr[:, hs], start=True, stop=True)
                    nc.scalar.activation(dtt[:, hs], pdt, AF.Exp,
                                         bias=bdt2[:, dtl:dtl + 1], scale=1.0)
                nc.scalar.activation(dtt, dtt, AF.Ln, bias=1.0, scale=1.0)
                dts.append(dtt)

            return xt, brep, crep, dts

        for ch in range(NCH):
            t0 = ch * TC
            xt, brep, crep, dts = frontend(ch)
            # ---- per (d-tile, group): the recurrence ----
            for dtl in range(NDT):
                pys = [ps_y.tile([128, PH], F32, tag="y", name=f"py{hf}")
                       for hf in range(TC // PH)]
                for q in range(GPD // 2):
                  for sub in (q, q + GPD // 2):
                    g = dtl * GPD + sub
                    rg = 0 if sub < GPD // 2 else 1
                    rsl = slice(rg * 64, rg * 64 + 64)

                    at = workp.tile([128, TC], F32)
                    gt = workp.tile([128, TC], F32)
                    for hf in range(TC // PH):
                        hs = slice(hf * PH, (hf + 1) * PH)
                        pdr = ps_rep.tile([128, PH], F32, tag="rep")
                        nc.tensor.matmul(pdr, sel128[rsl, sub, :],
                                         dts[dtl][rsl, hs],
                                         start=True, stop=True,
                                         tile_position=(rg * 64, 0))
                        nc.scalar.activation(at[:, hs], pdr, AF.Exp,
                                             scale=arep[:, g:g + 1])
                        pxrep = ps_rep.tile([128, PH], F32, tag="rep")
                        nc.tensor.matmul(pxrep, sel128[rsl, sub, :],
                                         xt[dtl][rsl, hs],
                                         start=True, stop=True,
                                         tile_position=(rg * 64, 0))
                        nc.vector.scalar_tensor_tensor(
                            gt[:, hs], pxrep, iarep[:, g:g + 1],
                            brep[:, hs], op0=AL.mult, op1=AL.mult)

                    ut = workp.tile([128, TC], F32)
                    nc.vector.scalar_tensor_tensor(
                        ut, at, -1.0, gt, op0=AL.add, op1=AL.mult)

                    st = scanp.tile([128, TC], BF16 if SC_BF16 else F32)
                    nc.vector.tensor_tensor_scan(
                        st, at, ut, carry[:, g:g + 1],
                        op0=AL.mult, op1=AL.add)
                    nc.scalar.copy(carry[:, g:g + 1], st[:, TC - 1:TC])

                    sct = scanp.tile([128, TC], BF16 if SC_BF16 else F32)
                    nc.vector.tensor_tensor(sct, st, crep, AL.mult)
                    for hf in range(TC // PH):
                        hs = slice(hf * PH, (hf + 1) * PH)
                        nc.tensor.matmul(pys[hf][rsl, :],
                                         nsum128[:, sub, rsl],
                                         sct[:, hs],
                                         start=(q == 0), stop=False,
                                         tile_position=(0, rg * 64))
                for hf in range(TC // PH):
                    hs = slice(hf * PH, (hf + 1) * PH)
                    nc.tensor.matmul(pys[hf], dskdiag[:, dtl, :],
                                     xt[dtl][:, hs],
                                     start=False, stop=True)

                yo = youtp.tile([128, TC], F32, tag="yo", name="yo")
                for hf in range(TC // PH):
                    nc.scalar.copy(yo[:, hf * PH:(hf + 1) * PH], pys[hf])
                nc.sync.dma_start(
                    y_d[dtl * 128:(dtl + 1) * 128, t0:t0 + TC], yo)


def _selectors():
    p = np.arange(128)
    k = np.arange(128)
    # sel128[s][k, p] = 1 iff k == s*8 + p//16  (replicate 8 rows over n)
    sel = np.stack([(k[:, None] == s * 8 + p[None, :] // 16)
                    for s in range(GPD)]).astype(np.float32)
    # nsum128[s][k, m] = 1 iff m == s*8 + k//16  (contract n into row block s)
    nsm = np.stack([(p[None, :] == s * 8 + k[:, None] // 16)
                    for s in range(GPD)]).astype(np.float32)
    # SBUF layout [k, s, p] flattened to [128, GPD*128]
    sel128 = np.ascontiguousarray(
        np.transpose(sel, (1, 0, 2)).reshape(128, GPD * 128))
    nsum128 = np.ascontiguousarray(
        np.transpose(nsm, (1, 0, 2)).reshape(128, GPD * 128))
    seln = (p[None, :] % 16 == np.arange(N)[:, None]).astype(np.float32)
    ident = np.eye(128, dtype=np.float32)
    return sel128, seln, nsum128, ident


def _dskdiag(dsk):
    """[512] -> [128, NDT*128]: per d-tile diagonal matrices, laid out
    [k, (d, p)] so sbuf tile [128, NDT, 128] slices to diag(dsk[dtl])."""
    out = np.zeros((128, NDT, 128), np.float32)
    for d in range(NDT):
        out[np.arange(128), d, np.arange(128)] = dsk[d * 128:(d + 1) * 128]
    return np.ascontiguousarray(out.reshape(128, NDT * 128))


def _rearr(m):
    """[512, 16] (d, n) -> [128, 64]: column g holds group g, row p=(d_sub*16+n)."""
    return np.ascontiguousarray(
        m.reshape(NG, 8, N).reshape(NG, 128).T)


def kernel(x, state, log_A, W_B, W_C, W_dt1, W_dt2, b_dt2, D_skip):
    if "nc" not in _CACHE:
        _CACHE["nc"] = _build_program()
    nc = _CACHE["nc"]

    x = np.asarray(x, np.float32)
    state = np.asarray(state, np.float32)
    A = (-np.exp(np.asarray(log_A, np.float32))).astype(np.float32)
    invA = (np.float32(1.0) / (A + np.float32(1e-8))).astype(np.float32)
    W_B = np.asarray(W_B, np.float32)
    W_C = np.asarray(W_C, np.float32)
    W_dt1 = np.asarray(W_dt1, np.float32)
    W_dt2 = np.asarray(W_dt2, np.float32)
    b_dt2 = np.asarray(b_dt2, np.float32)
    D_skip = np.asarray(D_skip, np.float32)

    sel128, seln, nsum128, ident = _selectors()

    in_maps = []
    for c in range(NCORES):
        b, h = c // 2, c % 2
        loc = slice(h * DH, (h + 1) * DH)
        oth = slice((1 - h) * DH, (2 - h) * DH)
        perm = np.r_[np.arange(h * DH, (h + 1) * DH),
                     np.arange((1 - h) * DH, (2 - h) * DH)]
        in_maps.append({
            "x": np.ascontiguousarray(x[b][:, perm]),
            "state_r": _rearr(state[b, loc]),
            "a_rep": _rearr(A[loc]),
            "inva_rep": _rearr(invA[loc]),
            "wb_t": np.ascontiguousarray(W_B.T[perm]),
            "wc_t": np.ascontiguousarray(W_C.T[perm]),
            "wdt1_t": np.ascontiguousarray(W_dt1.T[perm]),
            "wdt2_t": np.ascontiguousarray(W_dt2[loc].T),
            "bdt2": np.ascontiguousarray(b_dt2[loc].reshape(NDT, 128).T),
            "dskdiag": _dskdiag(D_skip[loc]),
            "sel128": sel128,
            "seln": seln,
            "nsum128": (nsum128.astype(ml_dtypes.bfloat16)
                        if SC_BF16 else nsum128),
            "ident": ident,
        })

    _CACHE["last_in_maps"] = in_maps
    res = run_bass_kernel_spmd(nc, in_maps, core_ids=list(range(NCORES)))

    y = np.empty((B, T, D), np.float32)
    for c in range(NCORES):
        b, h = c // 2, c % 2
        y[b][:, h * DH:(h + 1) * DH] = res.results[c]["yT"].T
    return y



# revision 18
# speedup vs baseline: 1.6781x; 1.6781x over previous
"""Mamba-1 style selective scan on 8 Trainium2 NeuronCores.

Sharding: core c -> (batch b = c//2, D-half h = c%2).  Each core receives
x[b] with its local 512 channels permuted to the front (weights permuted to
match), computes y^T[512, T] for its channels, host reassembles.

On-chip layout: partitions = (d_sub in 0..7) x (n in 0..15) "groups" of
8 channels x 16 states; free dim = time (chunks of TC).

Recurrence (v-shift form): with g_t = x_t*B_t/A and v_t := s_t + g_{t+1},
   v_t = A_bar_t * v_{t-1} + (g_{t+1} - g_t)
   y_t = sum_n C_t*v_t - x_{t+1} * K_t + D_skip*x_t,
   K_t = sum_n (1/A)*B_{t+1}*C_t   (PE matmul, contracted over n)
so the scan's additive input is a plain shifted difference of
w_t := g_{t+1} (no (A_bar-1)*g product, no expm1 pass).

Engines (fp16 data, f32 decay/PSUM): PE replicates dt/x across
state-partitions (fp16 selectors) and contracts y over n; ACT evaluates
exp and the invA-scaled PSUM->SBUF copies; DVE builds w/dg, runs all
scans and the PSUM-side y fixups; Pool (gpsimd) does the C-multiplies
and tail bookkeeping.  The group loop is software-pipelined with a
2-group skew so the PE->ACT->DVE->Pool chains of consecutive groups
overlap.
"""

import sys

import numpy as np

sys.path.insert(0, "/opt/trn_rl_repo")

import ml_dtypes

import concourse.bacc as bacc
import concourse.mybir as mybir
import concourse.tile as tile
from concourse.bass_utils import run_bass_kernel_spmd

B, T, D, N, R = 4, 4096, 1024, 16, 64
NCORES = 8
DH = D // 2            # channels per core
TC = 1024              # time chunk
XT = TC + 128          # xt tile width (one extra transpose block for t+1)
PH = 512               # PSUM half (one bank of f32)
NCH = T // TC
NDT = DH // 128        # 128-channel tiles per core (4)
NG = DH * N // 128     # (d,n) partition groups per core (64)
GPD = NG // NDT        # groups per d-tile (16)
F32 = mybir.dt.float32
BF16 = mybir.dt.bfloat16
AL = mybir.AluOpType
AF = mybir.ActivationFunctionType

# groups with (sub % 8) < SCAN_DVE_MOD run their C-multiply on DVE, rest Pool
SCAN_DVE_MOD = 6

_CACHE = {}


def _patch_act_tables():
    """Make the act-table pass pick natural_log_exp_and_others for Exp+Ln
    (same table indices; strip Exp/Ln from the single-func tables so the
    combined one is the only candidate -> no per-chunk LUT reload ping-pong)."""
    import concourse.bacc as _bacc
    from concourse.hw_specs import get_activation_tables as _orig

    def patched(arch):
        t = _orig(arch)
        exp = mybir.ActivationFunctionType.Exp
        ln = mybir.ActivationFunctionType.Ln
        for name, fns in t.items():
            if name != "natural_log_exp_and_others":
                fns.discard(exp)
                fns.discard(ln)
        return t

    _bacc.get_activation_tables = patched


def _build_program():
    _patch_act_tables()
    nc = bacc.Bacc(
        "TRN2",
        target_bir_lowering=False,
        debug=False,
        num_devices=NCORES,
    )

    x_d = nc.dram_tensor("x", [T, D], F32, kind="ExternalInput")
    vt_d = nc.dram_tensor("vtail0", [128, NG], F32, kind="ExternalInput")
    wt_d = nc.dram_tensor("wtail0", [128, NG], F32, kind="ExternalInput")
    a_d = nc.dram_tensor("a_rep", [128, NG], F32, kind="ExternalInput")
    ia_d = nc.dram_tensor("inva_rep", [128, NG], F32, kind="ExternalInput")
    wb_d = nc.dram_tensor("wb_t", [D, N], BF16, kind="ExternalInput")
    wc_d = nc.dram_tensor("wc_t", [D, N], BF16, kind="ExternalInput")
    w1_d = nc.dram_tensor("wdt1_t", [D, R], BF16, kind="ExternalInput")
    w2_d = nc.dram_tensor("wdt2_t", [R, DH], BF16, kind="ExternalInput")
    bd_d = nc.dram_tensor("bdt2", [128, NDT], F32, kind="ExternalInput")
    dsk_d = nc.dram_tensor("dskdiag", [128, NDT * 128], BF16,
                           kind="ExternalInput")
    sel8_d = nc.dram_tensor("sel128", [128, GPD * 128], BF16,
                            kind="ExternalInput")
    seln_d = nc.dram_tensor("seln", [N, 128], BF16, kind="ExternalInput")
    nsum_d = nc.dram_tensor("nsum128", [128, GPD * 128], BF16,
                            kind="ExternalInput")
    iak_d = nc.dram_tensor("iak", [N, DH], BF16, kind="ExternalInput")
    ident_d = nc.dram_tensor("ident", [128, 128], F32, kind="ExternalInput")
    y_d = nc.dram_tensor("yT", [DH, T], F32, kind="ExternalOutput")

    with tile.TileContext(nc) as tc:
        _body(tc, x_d, vt_d, wt_d, a_d, ia_d, wb_d, wc_d, w1_d, w2_d, bd_d,
              dsk_d, sel8_d, seln_d, nsum_d, iak_d, ident_d, y_d)

    nc.compile()
    return nc


def _body(tc, x_d, vt_d, wt_d, a_d, ia_d, wb_d, wc_d, w1_d, w2_d, bd_d,
          dsk_d, sel8_d, seln_d, nsum_d, iak_d, ident_d, y_d):
    nc = tc.nc
    KD = D // 128  # k-tiles over full D for the projections (8)

    with (
        tc.tile_pool(name="const", bufs=1) as const,
        tc.tile_pool(name="xload", bufs=3) as xload,
        tc.tile_pool(name="xt", bufs=2 * KD + 1) as xtp,
        tc.tile_pool(name="proj", bufs=2) as projp,
        tc.tile_pool(name="dtp", bufs=NDT) as dtp,
        tc.tile_pool(name="ksb", bufs=NDT + 1) as ksbp,
        tc.tile_pool(name="rep", bufs=2) as repp,
        tc.tile_pool(name="work", bufs=3) as workp,
        tc.tile_pool(name="scan", bufs=4) as scanp,
        tc.tile_pool(name="yout", bufs=3) as youtp,
        tc.tile_pool(name="ps_t", bufs=1, space="PSUM") as ps_t,
        tc.tile_pool(name="ps_proj", bufs=2, space="PSUM") as ps_proj,
        tc.tile_pool(name="ps_rep", bufs=4, space="PSUM") as ps_rep,
        tc.tile_pool(name="ps_y", bufs=2, space="PSUM") as ps_y,
    ):
        # ---- constants / small state ----
        ident = const.tile([128, 128], F32)
        nc.sync.dma_start(ident, ident_d[:, :])
        sel128 = const.tile([128, GPD, 128], BF16)
        nc.sync.dma_start(sel128, sel8_d.ap().rearrange("k (s p) -> k s p", s=GPD))
        seln = const.tile([N, 128], BF16)
        nc.sync.dma_start(seln, seln_d[:, :])
        nsum128 = const.tile([128, GPD, 128], BF16)
        nc.sync.dma_start(nsum128, nsum_d.ap().rearrange("k (s p) -> k s p", s=GPD))
        arep = const.tile([128, NG], F32)
        nc.sync.dma_start(arep, a_d[:, :])
        iarep = const.tile([128, NG], F32)
        nc.sync.dma_start(iarep, ia_d[:, :])
        bdt2 = const.tile([128, NDT], F32)
        nc.sync.dma_start(bdt2, bd_d[:, :])
        dskdiag = const.tile([128, NDT, 128], BF16)
        nc.sync.dma_start(
            dskdiag, dsk_d.ap().rearrange("k (d p) -> k d p", d=NDT))
        iak = const.tile([N, NDT, 128], BF16)
        nc.sync.dma_start(iak, iak_d.ap().rearrange("n (d p) -> n d p", d=NDT))
        vtail = const.tile([128, NG], F32)
        nc.sync.dma_start(vtail, vt_d[:, :])
        wtail = const.tile([128, NG], F32)
        nc.sync.dma_start(wtail, wt_d[:, :])

        wb = const.tile([128, KD, N], BF16)
        nc.sync.dma_start(wb, wb_d.ap().rearrange("(k p) n -> p k n", p=128))
        wc = const.tile([128, KD, N], BF16)
        nc.sync.dma_start(wc, wc_d.ap().rearrange("(k p) n -> p k n", p=128))
        w1 = const.tile([128, KD, R], BF16)
        nc.sync.dma_start(w1, w1_d.ap().rearrange("(k p) r -> p k r", p=128))
        w2 = const.tile([R, DH], BF16)
        nc.sync.dma_start(w2, w2_d[:, :])

        def frontend(ch):
            t0 = ch * TC
            # ---- load x chunk and transpose: xt[db] = x[t0:t0+TC+128, :].T
            # (bf16; the last 128 cols are the first block of the next chunk,
            # zeros for the final chunk) ----
            xt = [xtp.tile([128, XT], BF16, tag="xt", name=f"xt{db}")
                  for db in range(KD)]
            for tp2 in range(TC // 256):
                xl = []
                for j in range(2):
                    tt = 2 * tp2 + j
                    xld = xload.tile([128, D], F32, tag="xld", name=f"xld{j}")
                    nc.sync.dma_start(
                        xld, x_d[t0 + tt * 128: t0 + (tt + 1) * 128, :])
                    xl.append(xld)
                for db in range(KD):
                    pt = ps_t.tile([128, 256], F32, tag="tp")
                    for j in range(2):
                        nc.tensor.transpose(
                            pt[:, j * 128:(j + 1) * 128],
                            xl[j][:, db * 128:(db + 1) * 128], ident
                        )
                    nc.scalar.copy(
                        xt[db][:, tp2 * 256:(tp2 + 1) * 256], pt)
            if ch < NCH - 1:
                xle = xload.tile([128, D], F32, tag="xld", name="xlde")
                nc.sync.dma_start(xle, x_d[t0 + TC: t0 + TC + 128, :])
                for db in range(KD):
                    pt = ps_t.tile([128, 256], F32, tag="tp")
                    nc.tensor.transpose(
                        pt[:, 0:128], xle[:, db * 128:(db + 1) * 128], ident)
                    nc.scalar.copy(xt[db][:, TC:XT], pt[:, 0:128])
            else:
                for db in range(KD):
                    nc.vector.memset(xt[db][:, TC:XT], 0.0)

            # ---- projections over full D: xr[r,t], Bt[n,t] (shifted +1),
            # Ct[n,t]; n-replicas brep/crep ----
            xr = projp.tile([R, TC], BF16)
            bt = projp.tile([N, TC], BF16)
            ct = projp.tile([N, TC], BF16)
            brep = repp.tile([128, TC], BF16)
            crep = repp.tile([128, TC], BF16)
            for hf in range(TC // PH):
                hs = slice(hf * PH, (hf + 1) * PH)
                hs1 = slice(hf * PH + 1, (hf + 1) * PH + 1)
                pxr = ps_proj.tile([R, PH], F32, tag="proj")
                for k in range(KD):
                    nc.tensor.matmul(pxr, w1[:, k, :], xt[k][:, hs],
                                     start=(k == 0), stop=(k == KD - 1))
                nc.scalar.copy(xr[:, hs], pxr)
                pb = ps_proj.tile([N, PH], F32, tag="proj")
                for k in range(KD):
                    nc.tensor.matmul(pb, wb[:, k, :], xt[k][:, hs1],
                                     start=(k == 0), stop=(k == KD - 1))
                nc.scalar.copy(bt[:, hs], pb)
                pc = ps_proj.tile([N, PH], F32, tag="proj")
                for k in range(KD):
                    nc.tensor.matmul(pc, wc[:, k, :], xt[k][:, hs],
                                     start=(k == 0), stop=(k == KD - 1))
                nc.scalar.copy(ct[:, hs], pc)
                prb = ps_rep.tile([128, PH], F32, tag="rep")
                nc.tensor.matmul(prb, seln, bt[:, hs], start=True, stop=True)
                nc.scalar.copy(brep[:, hs], prb)
                prc = ps_rep.tile([128, PH], F32, tag="rep")
                nc.tensor.matmul(prc, seln, ct[:, hs], start=True, stop=True)
                nc.scalar.copy(crep[:, hs], prc)

            # ---- K[d,t] = sum_n invA[d,n]*B[n,t+1]*C[n,t] per d-tile ----
            bc = projp.tile([N, TC], BF16)
            nc.vector.tensor_tensor(bc, bt, ct, AL.mult)
            ksb = []
            for dtl in range(NDT):
                kt = ksbp.tile([128, TC], BF16, tag="ksb", name=f"ksb{dtl}")
                for hf in range(TC // PH):
                    hs = slice(hf * PH, (hf + 1) * PH)
                    pk = ps_proj.tile([128, PH], F32, tag="proj")
                    nc.tensor.matmul(pk, iak[:, dtl, :], bc[:, hs],
                                     start=True, stop=True)
                    nc.scalar.copy(kt[:, hs], pk)
                ksb.append(kt)

            # ---- dt per d-tile: softplus(W2 @ xr + b) ----
            dts = []
            for dtl in range(NDT):
                dtt = dtp.tile([128, TC], BF16, tag="dtt", name=f"dtt{dtl}")
                for hf in range(TC // PH):
                    hs = slice(hf * PH, (hf + 1) * PH)
                    pdt = ps_proj.tile([128, PH], F32, tag="proj")
                    nc.tensor.matmul(pdt, w2[:, dtl * 128:(dtl + 1) * 128],
                                     xr[:, hs], start=True, stop=True)
                    nc.scalar.activation(dtt[:, hs], pdt, AF.Exp,
                                         bias=bdt2[:, dtl:dtl + 1], scale=1.0)
                nc.scalar.activation(dtt, dtt, AF.Ln, bias=1.0, scale=1.0)
                dts.append(dtt)

            return xt, brep, crep, dts, ksb

        for ch in range(NCH):
            t0 = ch * TC
            xt, brep, crep, dts, ksb = frontend(ch)
            # ---- per (d-tile, group): the recurrence ----
            for dtl in range(NDT):
                pys = [ps_y.tile([128, PH], F32, tag="y", name=f"py{hf}")
                       for hf in range(TC // PH)]
                for q in range(GPD // 2):
                  for sub in (q, q + GPD // 2):
                    g = dtl * GPD + sub
                    rg = 0 if sub < GPD // 2 else 1
                    rsl = slice(rg * 64, rg * 64 + 64)

                    at = workp.tile([128, TC], F32, tag="at")
                    w = workp.tile([128, TC], F32, tag="w")
                    for hf in range(TC // PH):
                        hs = slice(hf * PH, (hf + 1) * PH)
                        hs1 = slice(hf * PH + 1, (hf + 1) * PH + 1)
                        pdr = ps_rep.tile([128, PH], F32, tag="rep")
                        nc.tensor.matmul(pdr, sel128[rsl, sub, :],
                                         dts[dtl][rsl, hs],
                                         start=True, stop=True,
                                         tile_position=(rg * 64, 0))
                        nc.scalar.activation(at[:, hs], pdr, AF.Exp,
                                             scale=arep[:, g:g + 1])
                        pxrep = ps_rep.tile([128, PH], F32, tag="rep")
                        nc.tensor.matmul(pxrep, sel128[rsl, sub, :],
                                         xt[dtl][rsl, hs1],
                                         start=True, stop=True,
                                         tile_position=(rg * 64, 0))
                        nc.vector.scalar_tensor_tensor(
                            w[:, hs], pxrep, iarep[:, g:g + 1],
                            brep[:, hs], op0=AL.mult, op1=AL.mult)

                    dg = workp.tile([128, TC], F32, tag="dg")
                    nc.gpsimd.tensor_tensor(
                        dg[:, 0:1], w[:, 0:1], wtail[:, g:g + 1], AL.subtract)
                    nc.gpsimd.tensor_tensor(
                        dg[:, 1:TC], w[:, 1:TC], w[:, 0:TC - 1], AL.subtract)
                    nc.gpsimd.tensor_scalar(
                        wtail[:, g:g + 1], w[:, TC - 1:TC], 1.0, None,
                        op0=AL.mult)

                    st = scanp.tile([128, TC], BF16, tag="st")
                    nc.vector.tensor_tensor_scan(
                        st, at, dg, vtail[:, g:g + 1],
                        op0=AL.mult, op1=AL.add)
                    nc.gpsimd.tensor_scalar(
                        vtail[:, g:g + 1], st[:, TC - 1:TC], 1.0, None,
                        op0=AL.mult)

                    sct = scanp.tile([128, TC], BF16, tag="sct")
                    if (sub % 8) < SCAN_DVE_MOD:
                        nc.vector.tensor_tensor(sct, st, crep, AL.mult)
                    else:
                        nc.gpsimd.tensor_tensor(sct, st, crep, AL.mult)
                    for hf in range(TC // PH):
                        hs = slice(hf * PH, (hf + 1) * PH)
                        nc.tensor.matmul(pys[hf][rsl, :],
                                         nsum128[:, sub, rsl],
                                         sct[:, hs],
                                         start=(q == 0), stop=False,
                                         tile_position=(0, rg * 64))
                for hf in range(TC // PH):
                    hs = slice(hf * PH, (hf + 1) * PH)
                    nc.tensor.matmul(pys[hf], dskdiag[:, dtl, :],
                                     xt[dtl][:, hs],
                                     start=False, stop=True)

                yk = youtp.tile([128, TC], BF16, tag="yk", name="yk")
                nc.gpsimd.tensor_tensor(
                    yk, xt[dtl][:, 1:TC + 1], ksb[dtl], AL.mult)
                yo = youtp.tile([128, TC], F32, tag="yo", name="yo")
                for hf in range(TC // PH):
                    hs = slice(hf * PH, (hf + 1) * PH)
                    nc.vector.tensor_tensor(
                        yo[:, hs], pys[hf], yk[:, hs], AL.subtract)
                nc.sync.dma_start(
                    y_d[dtl * 128:(dtl + 1) * 128, t0:t0 + TC], yo)


def _selectors():
    p = np.arange(128)
    k = np.arange(128)
    # sel128[s][k, p] = 1 iff k == s*8 + p//16  (replicate 8 rows over n)
    sel = np.stack([(k[:, None] == s * 8 + p[None, :] // 16)
                    for s in range(GPD)]).astype(np.float32)
    # nsum128[s][k, m] = 1 iff m == s*8 + k//16  (contract n into row block s)
    nsm = np.stack([(p[None, :] == s * 8 + k[:, None] // 16)
                    for s in range(GPD)]).astype(np.float32)
    # SBUF layout [k, s, p] flattened to [128, GPD*128]
    sel128 = np.ascontiguousarray(
        np.transpose(sel, (1, 0, 2)).reshape(128, GPD * 128))
    nsum128 = np.ascontiguousarray(
        np.transpose(nsm, (1, 0, 2)).reshape(128, GPD * 128))
    seln = (p[None, :] % 16 == np.arange(N)[:, None]).astype(np.float32)
    ident = np.eye(128, dtype=np.float32)
    return sel128, seln, nsum128, ident


def _dskdiag(dsk):
    """[512] -> [128, NDT*128]: per d-tile diagonal matrices, laid out
    [k, (d, p)] so sbuf tile [128, NDT, 128] slices to diag(dsk[dtl])."""
    out = np.zeros((128, NDT, 128), np.float32)
    for d in range(NDT):
        out[np.arange(128), d, np.arange(128)] = dsk[d * 128:(d + 1) * 128]
    return np.ascontiguousarray(out.reshape(128, NDT * 128))


def _rearr(m):
    """[512, 16] (d, n) -> [128, 64]: column g holds group g, row p=(d_sub*16+n)."""
    return np.ascontiguousarray(
        m.reshape(NG, 8, N).reshape(NG, 128).T)


def _bf(a):
    return np.ascontiguousarray(a).astype(ml_dtypes.bfloat16)


def kernel(x, state, log_A, W_B, W_C, W_dt1, W_dt2, b_dt2, D_skip):
    if "nc" not in _CACHE:
        _CACHE["nc"] = _build_program()
    nc = _CACHE["nc"]

    x = np.asarray(x, np.float32)
    state = np.asarray(state, np.float32)
    A = (-np.exp(np.asarray(log_A, np.float32))).astype(np.float32)
    invA = (np.float32(1.0) / (A + np.float32(1e-8))).astype(np.float32)
    W_B = np.asarray(W_B, np.float32)
    W_C = np.asarray(W_C, np.float32)
    W_dt1 = np.asarray(W_dt1, np.float32)
    W_dt2 = np.asarray(W_dt2, np.float32)
    b_dt2 = np.asarray(b_dt2, np.float32)
    D_skip = np.asarray(D_skip, np.float32)

    sel128, seln, nsum128, ident = _selectors()

    in_maps = []
    for c in range(NCORES):
        b, h = c // 2, c % 2
        loc = slice(h * DH, (h + 1) * DH)
        perm = np.r_[np.arange(h * DH, (h + 1) * DH),
                     np.arange((1 - h) * DH, (2 - h) * DH)]
        # g0[d, n] = invA * x_0[d] * B_0[n];  v_{-1} = s_{-1} + g_0
        B0 = W_B @ x[b, 0]                                       # [N]
        g0 = invA[loc] * x[b, 0, loc][:, None] * B0[None, :]     # [DH, N]
        in_maps.append({
            "x": np.ascontiguousarray(x[b][:, perm]),
            "vtail0": _rearr(state[b, loc] + g0),
            "wtail0": _rearr(g0),
            "a_rep": _rearr(A[loc]),
            "inva_rep": _rearr(invA[loc]),
            "wb_t": _bf(W_B.T[perm]),
            "wc_t": _bf(W_C.T[perm]),
            "wdt1_t": _bf(W_dt1.T[perm]),
            "wdt2_t": _bf(W_dt2[loc].T),
            "bdt2": np.ascontiguousarray(b_dt2[loc].reshape(NDT, 128).T),
            "dskdiag": _bf(_dskdiag(D_skip[loc])),
            "sel128": _bf(sel128),
            "seln": _bf(seln),
            "nsum128": _bf(nsum128),
            "iak": _bf(invA[loc].T),
            "ident": ident,
        })

    _CACHE["last_in_maps"] = in_maps
    res = run_bass_kernel_spmd(nc, in_maps, core_ids=list(range(NCORES)))

    y = np.empty((B, T, D), np.float32)
    for c in range(NCORES):
        b, h = c // 2, c % 2
        y[b][:, h * DH:(h + 1) * DH] = res.results[c]["yT"].T
    return y


# revision 19
# speedup vs baseline: 1.7197x; 1.0248x over previous
"""Mamba-1 style selective scan on 8 Trainium2 NeuronCores.

Sharding: core c -> (batch b = c//2, D-half h = c%2).  Each core receives
x[b] with its local 512 channels permuted to the front (weights permuted to
match), computes y^T[512, T] for its channels, host reassembles.

On-chip layout: partitions = (d_sub in 0..7) x (n in 0..15) "groups" of
8 channels x 16 states; free dim = time (chunks of TC).

Recurrence (v-shift form): with g_t = x_t*B_t/A and v_t := s_t + g_{t+1},
   v_t = A_bar_t * v_{t-1} + (g_{t+1} - g_t)
   y_t = sum_n C_t*v_t - x_{t+1} * K_t + D_skip*x_t,
   K_t = sum_n (1/A)*B_{t+1}*C_t   (PE matmul, contracted over n)
so the scan's additive input is a plain shifted difference of
w_t := g_{t+1} (no (A_bar-1)*g product, no expm1 pass).

Engines (fp16 data, f32 decay/PSUM): PE replicates dt/x across
state-partitions (fp16 selectors) and contracts y over n; ACT evaluates
exp and the invA-scaled PSUM->SBUF copies; DVE builds w/dg, runs all
scans and the PSUM-side y fixups; Pool (gpsimd) does the C-multiplies
and tail bookkeeping.  The group loop is software-pipelined with a
2-group skew so the PE->ACT->DVE->Pool chains of consecutive groups
overlap.
"""

import sys

import numpy as np

sys.path.insert(0, "/opt/trn_rl_repo")

import ml_dtypes

import concourse.bacc as bacc
import concourse.mybir as mybir
import concourse.tile as tile
from concourse.bass_utils import run_bass_kernel_spmd

B, T, D, N, R = 4, 4096, 1024, 16, 64
NCORES = 8
DH = D // 2            # channels per core
TC = 1024              # time chunk
XT = TC + 128          # xt tile width (one extra transpose block for t+1)
PH = 512               # PSUM half (one bank of f32)
NCH = T // TC
NDT = DH // 128        # 128-channel tiles per core (4)
NG = DH * N // 128     # (d,n) partition groups per core (64)
GPD = NG // NDT        # groups per d-tile (16)
F32 = mybir.dt.float32
BF16 = mybir.dt.bfloat16
AL = mybir.AluOpType
AF = mybir.ActivationFunctionType

# groups with (sub % 8) < SCAN_DVE_MOD run their C-multiply on DVE, rest Pool
SCAN_DVE_MOD = 6

_CACHE = {}


def _patch_act_tables():
    """Make the act-table pass pick natural_log_exp_and_others for Exp+Ln
    (same table indices; strip Exp/Ln from the single-func tables so the
    combined one is the only candidate -> no per-chunk LUT reload ping-pong)."""
    import concourse.bacc as _bacc
    from concourse.hw_specs import get_activation_tables as _orig

    def patched(arch):
        t = _orig(arch)
        exp = mybir.ActivationFunctionType.Exp
        ln = mybir.ActivationFunctionType.Ln
        for name, fns in t.items():
            if name != "natural_log_exp_and_others":
                fns.discard(exp)
                fns.discard(ln)
        return t

    _bacc.get_activation_tables = patched


def _build_program():
    _patch_act_tables()
    nc = bacc.Bacc(
        "TRN2",
        target_bir_lowering=False,
        debug=False,
        num_devices=NCORES,
    )

    x_d = nc.dram_tensor("x", [T, D], F32, kind="ExternalInput")
    vt_d = nc.dram_tensor("vtail0", [128, NG], F32, kind="ExternalInput")
    wt_d = nc.dram_tensor("wtail0", [128, NG], F32, kind="ExternalInput")
    a_d = nc.dram_tensor("a_rep", [128, NG], F32, kind="ExternalInput")
    ia_d = nc.dram_tensor("inva_rep", [128, NG], F32, kind="ExternalInput")
    wb_d = nc.dram_tensor("wb_t", [D, N], BF16, kind="ExternalInput")
    wc_d = nc.dram_tensor("wc_t", [D, N], BF16, kind="ExternalInput")
    w1_d = nc.dram_tensor("wdt1_t", [D, R], BF16, kind="ExternalInput")
    w2_d = nc.dram_tensor("wdt2_t", [R, DH], BF16, kind="ExternalInput")
    bd_d = nc.dram_tensor("bdt2", [128, NDT], F32, kind="ExternalInput")
    dsk_d = nc.dram_tensor("dskdiag", [128, NDT * 128], BF16,
                           kind="ExternalInput")
    sel8_d = nc.dram_tensor("sel128", [128, GPD * 128], BF16,
                            kind="ExternalInput")
    seln_d = nc.dram_tensor("seln", [N, 128], BF16, kind="ExternalInput")
    nsum_d = nc.dram_tensor("nsum128", [128, GPD * 128], BF16,
                            kind="ExternalInput")
    iak_d = nc.dram_tensor("iak", [N, DH], BF16, kind="ExternalInput")
    ident_d = nc.dram_tensor("ident", [128, 128], F32, kind="ExternalInput")
    y_d = nc.dram_tensor("yT", [DH, T], F32, kind="ExternalOutput")

    with tile.TileContext(nc) as tc:
        _body(tc, x_d, vt_d, wt_d, a_d, ia_d, wb_d, wc_d, w1_d, w2_d, bd_d,
              dsk_d, sel8_d, seln_d, nsum_d, iak_d, ident_d, y_d)

    nc.compile()
    return nc


def _body(tc, x_d, vt_d, wt_d, a_d, ia_d, wb_d, wc_d, w1_d, w2_d, bd_d,
          dsk_d, sel8_d, seln_d, nsum_d, iak_d, ident_d, y_d):
    nc = tc.nc
    KD = D // 128  # k-tiles over full D for the projections (8)

    with (
        tc.tile_pool(name="const", bufs=1) as const,
        tc.tile_pool(name="xload", bufs=3) as xload,
        tc.tile_pool(name="xt", bufs=2 * KD + 1) as xtp,
        tc.tile_pool(name="proj", bufs=2) as projp,
        tc.tile_pool(name="dtp", bufs=NDT) as dtp,
        tc.tile_pool(name="ksb", bufs=NDT + 1) as ksbp,
        tc.tile_pool(name="rep", bufs=2) as repp,
        tc.tile_pool(name="work", bufs=3) as workp,
        tc.tile_pool(name="scan", bufs=4) as scanp,
        tc.tile_pool(name="yout", bufs=3) as youtp,
        tc.tile_pool(name="ps_t", bufs=1, space="PSUM") as ps_t,
        tc.tile_pool(name="ps_proj", bufs=2, space="PSUM") as ps_proj,
        tc.tile_pool(name="ps_rep", bufs=4, space="PSUM") as ps_rep,
        tc.tile_pool(name="ps_y", bufs=2, space="PSUM") as ps_y,
    ):
        # ---- constants / small state ----
        ident = const.tile([128, 128], F32)
        nc.sync.dma_start(ident, ident_d[:, :])
        sel128 = const.tile([128, GPD, 128], BF16)
        nc.sync.dma_start(sel128, sel8_d.ap().rearrange("k (s p) -> k s p", s=GPD))
        seln = const.tile([N, 128], BF16)
        nc.sync.dma_start(seln, seln_d[:, :])
        nsum128 = const.tile([128, GPD, 128], BF16)
        nc.sync.dma_start(nsum128, nsum_d.ap().rearrange("k (s p) -> k s p", s=GPD))
        arep = const.tile([128, NG], F32)
        nc.sync.dma_start(arep, a_d[:, :])
        iarep = const.tile([128, NG], F32)
        nc.sync.dma_start(iarep, ia_d[:, :])
        bdt2 = const.tile([128, NDT], F32)
        nc.sync.dma_start(bdt2, bd_d[:, :])
        dskdiag = const.tile([128, NDT, 128], BF16)
        nc.sync.dma_start(
            dskdiag, dsk_d.ap().rearrange("k (d p) -> k d p", d=NDT))
        iak = const.tile([N, NDT, 128], BF16)
        nc.sync.dma_start(iak, iak_d.ap().rearrange("n (d p) -> n d p", d=NDT))
        vtail = const.tile([128, NG], F32)
        nc.sync.dma_start(vtail, vt_d[:, :])
        wtail = const.tile([128, NG], F32)
        nc.sync.dma_start(wtail, wt_d[:, :])

        wb = const.tile([128, KD, N], BF16)
        nc.sync.dma_start(wb, wb_d.ap().rearrange("(k p) n -> p k n", p=128))
        wc = const.tile([128, KD, N], BF16)
        nc.sync.dma_start(wc, wc_d.ap().rearrange("(k p) n -> p k n", p=128))
        w1 = const.tile([128, KD, R], BF16)
        nc.sync.dma_start(w1, w1_d.ap().rearrange("(k p) r -> p k r", p=128))
        w2 = const.tile([R, DH], BF16)
        nc.sync.dma_start(w2, w2_d[:, :])

        def frontend(ch):
            t0 = ch * TC
            # ---- load x chunk and transpose: xt[db] = x[t0:t0+TC+128, :].T
            # (bf16; the last 128 cols are the first block of the next chunk,
            # zeros for the final chunk) ----
            xt = [xtp.tile([128, XT], BF16, tag="xt", name=f"xt{db}")
                  for db in range(KD)]
            for tp2 in range(TC // 256):
                xl = []
                for j in range(2):
                    tt = 2 * tp2 + j
                    xld = xload.tile([128, D], F32, tag="xld", name=f"xld{j}")
                    nc.sync.dma_start(
                        xld, x_d[t0 + tt * 128: t0 + (tt + 1) * 128, :])
                    xl.append(xld)
                for db in range(KD):
                    pt = ps_t.tile([128, 256], F32, tag="tp")
                    for j in range(2):
                        nc.tensor.transpose(
                            pt[:, j * 128:(j + 1) * 128],
                            xl[j][:, db * 128:(db + 1) * 128], ident
                        )
                    nc.scalar.copy(
                        xt[db][:, tp2 * 256:(tp2 + 1) * 256], pt)
            if ch < NCH - 1:
                xle = xload.tile([128, D], F32, tag="xld", name="xlde")
                nc.sync.dma_start(xle, x_d[t0 + TC: t0 + TC + 128, :])
                for db in range(KD):
                    pt = ps_t.tile([128, 256], F32, tag="tp")
                    nc.tensor.transpose(
                        pt[:, 0:128], xle[:, db * 128:(db + 1) * 128], ident)
                    nc.scalar.copy(xt[db][:, TC:XT], pt[:, 0:128])
            else:
                for db in range(KD):
                    nc.vector.memset(xt[db][:, TC:XT], 0.0)

            # ---- projections over full D: xr[r,t], Bt[n,t] (shifted +1),
            # Ct[n,t]; n-replicas brep/crep ----
            xr = projp.tile([R, TC], BF16)
            bt = projp.tile([N, TC], BF16)
            ct = projp.tile([N, TC], BF16)
            brep = repp.tile([128, TC], BF16)
            crep = repp.tile([128, TC], BF16)
            for hf in range(TC // PH):
                hs = slice(hf * PH, (hf + 1) * PH)
                hs1 = slice(hf * PH + 1, (hf + 1) * PH + 1)
                pxr = ps_proj.tile([R, PH], F32, tag="proj")
                for k in range(KD):
                    nc.tensor.matmul(pxr, w1[:, k, :], xt[k][:, hs],
                                     start=(k == 0), stop=(k == KD - 1))
                nc.scalar.copy(xr[:, hs], pxr)
                pb = ps_proj.tile([N, PH], F32, tag="proj")
                for k in range(KD):
                    nc.tensor.matmul(pb, wb[:, k, :], xt[k][:, hs1],
                                     start=(k == 0), stop=(k == KD - 1))
                nc.scalar.copy(bt[:, hs], pb)
                pc = ps_proj.tile([N, PH], F32, tag="proj")
                for k in range(KD):
                    nc.tensor.matmul(pc, wc[:, k, :], xt[k][:, hs],
                                     start=(k == 0), stop=(k == KD - 1))
                nc.scalar.copy(ct[:, hs], pc)
                prb = ps_proj.tile([128, PH], F32, tag="proj")
                nc.tensor.matmul(prb, seln, bt[:, hs], start=True, stop=True)
                nc.scalar.copy(brep[:, hs], prb)
                prc = ps_proj.tile([128, PH], F32, tag="proj")
                nc.tensor.matmul(prc, seln, ct[:, hs], start=True, stop=True)
                nc.scalar.copy(crep[:, hs], prc)

            # ---- K[d,t] = sum_n invA[d,n]*B[n,t+1]*C[n,t] per d-tile ----
            bc = projp.tile([N, TC], BF16)
            nc.vector.tensor_tensor(bc, bt, ct, AL.mult)
            ksb = []
            for dtl in range(NDT):
                kt = ksbp.tile([128, TC], BF16, tag="ksb", name=f"ksb{dtl}")
                for hf in range(TC // PH):
                    hs = slice(hf * PH, (hf + 1) * PH)
                    pk = ps_proj.tile([128, PH], F32, tag="proj")
                    nc.tensor.matmul(pk, iak[:, dtl, :], bc[:, hs],
                                     start=True, stop=True)
                    nc.scalar.copy(kt[:, hs], pk)
                ksb.append(kt)

            # ---- dt per d-tile: softplus(W2 @ xr + b) ----
            dts = []
            for dtl in range(NDT):
                dtt = dtp.tile([128, TC], BF16, tag="dtt", name=f"dtt{dtl}")
                for hf in range(TC // PH):
                    hs = slice(hf * PH, (hf + 1) * PH)
                    pdt = ps_proj.tile([128, PH], F32, tag="proj")
                    nc.tensor.matmul(pdt, w2[:, dtl * 128:(dtl + 1) * 128],
                                     xr[:, hs], start=True, stop=True)
                    nc.scalar.activation(dtt[:, hs], pdt, AF.Exp,
                                         bias=bdt2[:, dtl:dtl + 1], scale=1.0)
                nc.scalar.activation(dtt, dtt, AF.Ln, bias=1.0, scale=1.0)
                dts.append(dtt)

            return xt, brep, crep, dts, ksb

        for ch in range(NCH):
            t0 = ch * TC
            xt, brep, crep, dts, ksb = frontend(ch)
            # ---- per (d-tile, group): the recurrence ----
            for dtl in range(NDT):
                pys = [ps_y.tile([128, PH], F32, tag="y", name=f"py{hf}")
                       for hf in range(TC // PH)]
                for q in range(GPD // 2):
                  for sub in (q, q + GPD // 2):
                    g = dtl * GPD + sub
                    rg = 0 if sub < GPD // 2 else 1
                    rsl = slice(rg * 64, rg * 64 + 64)

                    at = workp.tile([128, TC], F32, tag="at")
                    w = workp.tile([128, TC], F32, tag="w")
                    for hf in range(TC // PH):
                        hs = slice(hf * PH, (hf + 1) * PH)
                        hs1 = slice(hf * PH + 1, (hf + 1) * PH + 1)
                        pdr = ps_rep.tile([128, PH], F32, tag="rep")
                        nc.tensor.matmul(pdr, sel128[rsl, sub, :],
                                         dts[dtl][rsl, hs],
                                         start=True, stop=True,
                                         tile_position=(rg * 64, 0))
                        nc.scalar.activation(at[:, hs], pdr, AF.Exp,
                                             scale=arep[:, g:g + 1])
                        pxrep = ps_rep.tile([128, PH], F32, tag="rep")
                        nc.tensor.matmul(pxrep, sel128[rsl, sub, :],
                                         xt[dtl][rsl, hs1],
                                         start=True, stop=True,
                                         tile_position=(rg * 64, 0))
                        nc.vector.scalar_tensor_tensor(
                            w[:, hs], pxrep, iarep[:, g:g + 1],
                            brep[:, hs], op0=AL.mult, op1=AL.mult)

                    dg = workp.tile([128, TC], F32, tag="dg")
                    nc.gpsimd.tensor_tensor(
                        dg[:, 0:1], w[:, 0:1], wtail[:, g:g + 1], AL.subtract)
                    nc.gpsimd.tensor_tensor(
                        dg[:, 1:TC], w[:, 1:TC], w[:, 0:TC - 1], AL.subtract)
                    nc.gpsimd.tensor_scalar(
                        wtail[:, g:g + 1], w[:, TC - 1:TC], 1.0, None,
                        op0=AL.mult)

                    st = scanp.tile([128, TC], BF16, tag="st")
                    nc.vector.tensor_tensor_scan(
                        st, at, dg, vtail[:, g:g + 1],
                        op0=AL.mult, op1=AL.add)
                    nc.gpsimd.tensor_scalar(
                        vtail[:, g:g + 1], st[:, TC - 1:TC], 1.0, None,
                        op0=AL.mult)

                    sct = scanp.tile([128, TC], BF16, tag="sct")
                    if (sub % 8) < SCAN_DVE_MOD:
                        nc.vector.tensor_tensor(sct, st, crep, AL.mult)
                    else:
                        nc.gpsimd.tensor_tensor(sct, st, crep, AL.mult)
                    for hf in range(TC // PH):
                        hs = slice(hf * PH, (hf + 1) * PH)
                        nc.tensor.matmul(pys[hf][rsl, :],
                                         nsum128[:, sub, rsl],
                                         sct[:, hs],
                                         start=(q == 0), stop=False,
                                         tile_position=(0, rg * 64))
                for hf in range(TC // PH):
                    hs = slice(hf * PH, (hf + 1) * PH)
                    nc.tensor.matmul(pys[hf], dskdiag[:, dtl, :],
                                     xt[dtl][:, hs],
                                     start=False, stop=True)

                yk = youtp.tile([128, TC], BF16, tag="yk", name="yk")
                nc.gpsimd.tensor_tensor(
                    yk, xt[dtl][:, 1:TC + 1], ksb[dtl], AL.mult)
                yo = youtp.tile([128, TC], F32, tag="yo", name="yo")
                for hf in range(TC // PH):
                    hs = slice(hf * PH, (hf + 1) * PH)
                    nc.vector.tensor_tensor(
                        yo[:, hs], pys[hf], yk[:, hs], AL.subtract)
                nc.sync.dma_start(
                    y_d[dtl * 128:(dtl + 1) * 128, t0:t0 + TC], yo)


def _selectors():
    p = np.arange(128)
    k = np.arange(128)
    # sel128[s][k, p] = 1 iff k == s*8 + p//16  (replicate 8 rows over n)
    sel = np.stack([(k[:, None] == s * 8 + p[None, :] // 16)
                    for s in range(GPD)]).astype(np.float32)
    # nsum128[s][k, m] = 1 iff m == s*8 + k//16  (contract n into row block s)
    nsm = np.stack([(p[None, :] == s * 8 + k[:, None] // 16)
                    for s in range(GPD)]).astype(np.float32)
    # SBUF layout [k, s, p] flattened to [128, GPD*128]
    sel128 = np.ascontiguousarray(
        np.transpose(sel, (1, 0, 2)).reshape(128, GPD * 128))
    nsum128 = np.ascontiguousarray(
        np.transpose(nsm, (1, 0, 2)).reshape(128, GPD * 128))
    seln = (p[None, :] % 16 == np.arange(N)[:, None]).astype(np.float32)
    ident = np.eye(128, dtype=np.float32)
    return sel128, seln, nsum128, ident


def _dskdiag(dsk):
    """[512] -> [128, NDT*128]: per d-tile diagonal matrices, laid out
    [k, (d, p)] so sbuf tile [128, NDT, 128] slices to diag(dsk[dtl])."""
    out = np.zeros((128, NDT, 128), np.float32)
    for d in range(NDT):
        out[np.arange(128), d, np.arange(128)] = dsk[d * 128:(d + 1) * 128]
    return np.ascontiguousarray(out.reshape(128, NDT * 128))


def _rearr(m):
    """[512, 16] (d, n) -> [128, 64]: column g holds group g, row p=(d_sub*16+n)."""
    return np.ascontiguousarray(
        m.reshape(NG, 8, N).reshape(NG, 128).T)


def _bf(a):
    return np.ascontiguousarray(a).astype(ml_dtypes.bfloat16)


def kernel(x, state, log_A, W_B, W_C, W_dt1, W_dt2, b_dt2, D_skip):
    if "nc" not in _CACHE:
        _CACHE["nc"] = _build_program()
    nc = _CACHE["nc"]

    x = np.asarray(x, np.float32)
    state = np.asarray(state, np.float32)
    A = (-np.exp(np.asarray(log_A, np.float32))).astype(np.float32)
    invA = (np.float32(1.0) / (A + np.float32(1e-8))).astype(np.float32)
    W_B = np.asarray(W_B, np.float32)
    W_C = np.asarray(W_C, np.float32)
    W_dt1 = np.asarray(W_dt1, np.float32)
    W_dt2 = np.asarray(W_dt2, np.float32)
    b_dt2 = np.asarray(b_dt2, np.float32)
    D_skip = np.asarray(D_skip, np.float32)

    sel128, seln, nsum128, ident = _selectors()

    in_maps = []
    for c in range(NCORES):
        b, h = c // 2, c % 2
        loc = slice(h * DH, (h + 1) * DH)
        perm = np.r_[np.arange(h * DH, (h + 1) * DH),
                     np.arange((1 - h) * DH, (2 - h) * DH)]
        # g0[d, n] = invA * x_0[d] * B_0[n];  v_{-1} = s_{-1} + g_0
        B0 = W_B @ x[b, 0]                                       # [N]
        g0 = invA[loc] * x[b, 0, loc][:, None] * B0[None, :]     # [DH, N]
        in_maps.append({
            "x": np.ascontiguousarray(x[b][:, perm]),
            "vtail0": _rearr(state[b, loc] + g0),
            "wtail0": _rearr(g0),
            "a_rep": _rearr(A[loc]),
            "inva_rep": _rearr(invA[loc]),
            "wb_t": _bf(W_B.T[perm]),
            "wc_t": _bf(W_C.T[perm]),
            "wdt1_t": _bf(W_dt1.T[perm]),
            "wdt2_t": _bf(W_dt2[loc].T),
            "bdt2": np.ascontiguousarray(b_dt2[loc].reshape(NDT, 128).T),
            "dskdiag": _bf(_dskdiag(D_skip[loc])),
            "sel128": _bf(sel128),
            "seln": _bf(seln),
            "nsum128": _bf(nsum128),
            "iak": _bf(invA[loc].T),
            "ident": ident,
        })

    _CACHE["last_in_maps"] = in_maps
    res = run_bass_kernel_spmd(nc, in_maps, core_ids=list(range(NCORES)))

    y = np.empty((B, T, D), np.float32)
    for c in range(NCORES):
        b, h = c // 2, c % 2
        y[b][:, h * DH:(h + 1) * DH] = res.results[c]["yT"].T
    return y
